# revision 31
# baseline (speedup 1.0000x reference)
"""2-layer GAT (PyG-style, eval mode) on 8 Trainium2 NeuronCores via Bass/Tile.

Architecture (dst-sharded, destination-bucketed, dma_gather based):
  - Destination nodes are sharded across 8 cores (12500 each, padded 12544).
  - Per core, dst nodes are packed into 98 blocks of 128 (partition dim) by
    recursive coordinate bisection on their per-group in-edge counts, so the
    per-(block, group) max-over-partition slot counts stay near the mean
    (slot padding ~1.29x vs 2.05x for plain degree sort).
  - Per-edge source rows ([a_s | xp], 256B-strided table) are fetched with
    InstDMAGatherAnt over 4 SWDGE queues (one per int16-addressable table
    quarter). Each core's index planes carry trailing -1 columns per
    (block, group); the Q7 ucode trims trailing negatives, so descriptor
    generation (the kernel's bottleneck) runs at each core's own padding
    even though instruction shapes are baked to the cross-core max. The
    cross-core slack columns are masked by memsetting their a_s lanes to
    -1e30 before the gather lands (=> p = 0).
  - e = a_s + a_d, leaky-relu, and exp all run on the Activation engine
    (Lrelu with bias/alpha, then Exp with accum_out producing the softmax
    denominator), keeping DVE for just the p*xp multiply + reduce.
  - Segment softmax folds into: p = exp(leaky_relu(a_s + a_d)); S = sum_j p;
    out = (sum_j p * xp) / S  (no segment-max: attention logits are O(1), so
    exp cannot overflow; this matches the reference up to fp rounding).
  - Node-feature tables are exchanged compactly with AllGather and expanded
    to the 256B-aligned gather layout on device with fat DMAs + DVE copies.
"""

import os
import sys

sys.path.insert(0, "/opt/trn_rl_repo")

import numpy as np

# bisect switches (default on; set GAT_NO_TRIM=1 / GAT_NO_ACTFUSE=1 to disable)
USE_TRIM = os.environ.get("GAT_NO_TRIM", "") != "1"
USE_ACTFUSE = os.environ.get("GAT_NO_ACTFUSE", "") != "1"

import concourse.bass as bass
import concourse.bacc as bacc
import concourse.mybir as mybir
from concourse.tile import TileContext
from concourse import library_config
from concourse.bass_utils import run_bass_kernel_spmd

F32 = mybir.dt.float32
I16 = mybir.dt.int16

AX = mybir.AxisListType.X
ALU = mybir.AluOpType
ACTF = mybir.ActivationFunctionType

NEG_BIG = -1.0e30


class Cfg:
    def __init__(self, N=100000, E=3200000, F_IN=512, ncores=8):
        self.N = N
        self.E = E
        self.F_IN = F_IN
        self.H1, self.C1 = 4, 8
        self.H2, self.C2 = 1, 8
        self.NEG_SLOPE = 0.2
        self.NCORES = ncores
        assert N % ncores == 0
        self.NPC = N // ncores                      # real nodes per core
        self.NPAD = ((self.NPC + 127) // 128) * 128  # padded
        self.NBLK = self.NPAD // 128
        self.GROUPS = 4
        assert (self.NPAD * ncores) % self.GROUPS == 0
        self.GROUP_ROWS = self.NPAD * ncores // self.GROUPS  # table rows/group
        assert self.GROUP_ROWS <= 32767
        # compact row layouts (fp32 words)
        self.ROW1C = 4 + self.H1 * self.C1   # 36: [a_s1(4) | xp1(32)]
        self.ROW2C = 1 + self.C2             # 9:  [a_s2(1) | xp2(8)]
        self.ROWP = 64                       # padded row: 256B


# ----------------------------------------------------------------------------
# Host-side preprocessing: sharding, packing, slot/idx construction
# ----------------------------------------------------------------------------
def _rcb_order(cnt, npad):
    """Recursive coordinate bisection on per-group counts; returns
    rank -> row index into cnt (rows >= len(cnt) are zero-profile pads)."""
    npc = cnt.shape[0]
    cntp = np.zeros((npad, cnt.shape[1]), cnt.dtype)
    cntp[:npc] = cnt
    out = []

    def bisect(ids):
        n = len(ids)
        if n <= 128:
            out.append(ids)
            return
        vals = cntp[ids]
        g = int(np.argmax(vals.std(0)))
        srt = np.argsort(vals[:, g], kind="stable")
        h = max(128, ((n // 2) // 128) * 128)
        bisect(ids[srt[:h]])
        bisect(ids[srt[h:]])

    bisect(np.arange(npad))
    return np.concatenate(out)


def _refine(node_at, cnt, nblk, deadline=3.0):
    """Local search: swap per-(block,group) argmax nodes into blocks where
    they fit under the maxima. Mutates and returns node_at ([npad] rank->row,
    pads included as zero-profile rows)."""
    import time

    t0 = time.time()
    npad = node_at.size
    prof = np.zeros((npad, cnt.shape[1]), cnt.dtype)
    real = node_at < cnt.shape[0]
    prof[real] = cnt[node_at[real]]
    P = prof.reshape(nblk, 128, -1)
    NA = node_at.reshape(nblk, 128)
    for _ in range(40):
        M = P.max(axis=1)
        swaps = 0
        for b in range(nblk):
            for g in range(P.shape[2]):
                col = P[b, :, g]
                mx = col.max()
                lower = col[col < mx]
                if lower.size == 0:
                    continue
                second = lower.max()
                if mx - second <= 0:
                    continue
                attain = np.where(col == mx)[0]
                if len(attain) > 2:
                    continue
                p = int(attain[0])
                cu = P[b, p].copy()
                fit_b = (cu[None, :] <= M).all(1)
                fit_b[b] = False
                for b2 in np.where(fit_b)[0]:
                    cand = (P[b2, :, g] <= second) & (
                        P[b2] <= M[b][None, :]).all(1)
                    if cand.any():
                        p2 = int(np.argmax(cand))
                        P[b, p], P[b2, p2] = P[b2, p2].copy(), P[b, p].copy()
                        NA[b, p], NA[b2, p2] = NA[b2, p2], NA[b, p]
                        M[b] = P[b].max(0)
                        M[b2] = P[b2].max(0)
                        swaps += 1
                        break
        if swaps == 0 or time.time() - t0 > deadline:
            break
    return node_at


class Plan:
    """Host plan: per-core node placement (rank -> node), slot structure
    (global shapes + per-core trailing trim) and gather index planes."""

    def __init__(self, cfg: Cfg, edge_index: np.ndarray):
        c = cfg
        src0 = edge_index[0].astype(np.int64)
        dst0 = edge_index[1].astype(np.int64)
        loop = np.arange(c.N, dtype=np.int64)
        src = np.concatenate([src0, loop])
        dst = np.concatenate([dst0, loop])

        grp_of_src_node = (src // c.NPC) // 2 if c.NCORES == 8 else \
            np.minimum((src // c.NPC) * c.GROUPS // c.NCORES, c.GROUPS - 1)

        core_of = dst // c.NPC
        # ---- per-core packing ----
        # node_at[ci][r] = local node at rank r, or -1 (pad)
        self.node_at = []
        per_core = []
        cnts = []
        for ci in range(c.NCORES):
            m = core_of == ci
            s_c, d_c = src[m], dst[m] - ci * c.NPC
            g_c = grp_of_src_node[m]
            cnt = np.zeros((c.NPC, c.GROUPS), np.int64)
            np.add.at(cnt, (d_c, g_c), 1)
            order = _rcb_order(cnt, c.NPAD)
            order = _refine(order, cnt, c.NBLK, deadline=6.0)
            na = np.where(order < c.NPC, order, -1)
            self.node_at.append(na)
            per_core.append((s_c, d_c))
            cnts.append(cnt)

        # global "rank" -> table row of any node (as a source):
        # node n (core k, local l) at rank r -> row k*NPAD + NBLK*(r%128)+r//128
        self.rank_of_global = np.empty(c.N, dtype=np.int64)
        self.pad_rows = []   # per core: table rows of pad ranks
        for ci in range(c.NCORES):
            na = self.node_at[ci]
            ranks = np.arange(c.NPAD)
            rc = c.NBLK * (ranks % 128) + ranks // 128
            realm = na >= 0
            self.rank_of_global[ci * c.NPC + na[realm]] = ci * c.NPAD + rc[realm]
            self.pad_rows.append(ci * c.NPAD + rc[~realm])

        tbl_row = self.rank_of_global
        # ---- per-core per-(block, group) counts ----
        self.Dk = np.zeros((c.NCORES, c.NBLK, c.GROUPS), np.int64)
        self.core_edges = []
        for ci in range(c.NCORES):
            s_c, d_c = per_core[ci]
            na = self.node_at[ci]
            rank_of = np.empty(c.NPC, np.int64)
            ranks = np.arange(c.NPAD)
            realm = na >= 0
            rank_of[na[realm]] = ranks[realm]
            r_c = rank_of[d_c]
            b = r_c // 128
            p = r_c % 128
            g = tbl_row[s_c] // c.GROUP_ROWS
            counts = np.zeros((c.NBLK, c.GROUPS, 128), np.int64)
            np.add.at(counts, (b, g, p), 1)
            self.Dk[ci] = counts.max(axis=2)
            self.core_edges.append((s_c, r_c, b, p, g))
        self.D = np.maximum(self.Dk.max(axis=0), 1)   # global shapes
        self.Dmin = self.Dk.min(axis=0)               # memset slack floor
        self.slots_bg = 128 * self.D
        self.tot_slots = int(self.slots_bg.sum())

        # dummy row per group: a pad row of core 2g (core-major table layout
        # puts cores 2g,2g+1 inside group g). Pad rows get a_s=-1e30 via the
        # per-block pad mask, so p = 0 for slots pointing at them.
        self.dummy_local = np.zeros(c.GROUPS, dtype=np.int64)
        for g in range(c.GROUPS):
            k = (g * c.GROUP_ROWS) // c.NPAD
            cand = [r for r in self.pad_rows[k]
                    if g * c.GROUP_ROWS <= r < (g + 1) * c.GROUP_ROWS]
            if not cand:
                for kk in range(c.NCORES):
                    cand = [r for r in self.pad_rows[kk]
                            if g * c.GROUP_ROWS <= r < (g + 1) * c.GROUP_ROWS]
                    if cand:
                        break
            assert cand, f"no pad row available for group {g}"
            self.dummy_local[g] = cand[0] - g * c.GROUP_ROWS

        # gather idx planes per core: concat over (b, g) of wrapped
        # [128, 8*D] int16 planes; trailing columns beyond this core's own
        # Dk are -1 (trimmed by the gather ucode => no descriptors).
        self.idx_planes = []
        for ci in range(c.NCORES):
            s_c, r_c, b, p, g = self.core_edges[ci]
            segs = []
            for bb in range(c.NBLK):
                for gg in range(c.GROUPS):
                    Dn = int(self.D[bb, gg])
                    Dkn = int(self.Dk[ci, bb, gg])
                    nslots = 128 * Dn
                    sel = (b == bb) & (g == gg)
                    pp = p[sel]
                    loc = tbl_row[s_c[sel]] - gg * c.GROUP_ROWS
                    ordr = np.argsort(pp, kind="stable")
                    pp_s = pp[ordr]
                    loc_s = loc[ordr]
                    jj = np.arange(pp_s.size) - np.searchsorted(pp_s, pp_s)
                    slot = jj * 128 + pp_s
                    arr = np.full(nslots, self.dummy_local[gg], dtype=np.int16)
                    arr[slot] = loc_s.astype(np.int16)
                    if USE_TRIM:
                        arr[128 * Dkn:] = -1
                    segs.append(arr.reshape(-1, 16).T)  # [16, nslots/16]
            wrapped = np.concatenate(segs, axis=1)
            plane = np.tile(wrapped, (8, 1)).astype(np.int16)
            self.idx_planes.append(plane)
        self.idx_cols = self.idx_planes[0].shape[1]


# ----------------------------------------------------------------------------
# Device kernel builder (one program, SPMD on 8 cores)
# ----------------------------------------------------------------------------
def build_kernel(cfg: Cfg, plan: Plan):
    c = cfg
    NB = c.NBLK
    TROWS = c.NPAD * c.NCORES          # padded table rows (100352)
    nc = bacc.Bacc(num_swdge_queues=4, num_devices=c.NCORES)

    # ---- inputs ----
    xT = nc.dram_tensor("xT", [c.F_IN, c.NPAD], F32, kind="ExternalInput")
    w1e = nc.dram_tensor("w1e", [c.F_IN, 40], F32, kind="ExternalInput")
    w2e = nc.dram_tensor("w2e", [32, 12], F32, kind="ExternalInput")
    b1r = nc.dram_tensor("b1r", [128, 32], F32, kind="ExternalInput")
    b2r = nc.dram_tensor("b2r", [128, 8], F32, kind="ExternalInput")
    padneg = nc.dram_tensor("padneg", [128, NB], F32, kind="ExternalInput")
    padone = nc.dram_tensor("padone", [128, NB], F32, kind="ExternalInput")
    idxt = nc.dram_tensor("idxt", [128, plan.idx_cols], I16, kind="ExternalInput")
    nidx = nc.dram_tensor("nidx", [1, NB * cfg.GROUPS], mybir.dt.int32,
                          kind="ExternalInput")
    y = nc.dram_tensor("y", [128, NB * 8], F32, kind="ExternalOutput")

    # ---- internal DRAM ----
    tc1_in = nc.dram_tensor("tc1_in", [128 * NB * c.ROW1C], F32, kind="Internal")
    tc1_full = nc.dram_tensor("tc1_full", [TROWS * c.ROW1C], F32,
                              kind="Internal", addr_space="Shared")
    tbl1 = nc.dram_tensor("tbl1", [TROWS, c.ROWP], F32, kind="Internal")
    tc2_in = nc.dram_tensor("tc2_in", [128 * NB * c.ROW2C], F32, kind="Internal")
    tc2_full = nc.dram_tensor("tc2_full", [TROWS * c.ROW2C], F32,
                              kind="Internal", addr_space="Shared")
    tbl2 = nc.dram_tensor("tbl2", [TROWS, c.ROWP], F32, kind="Internal")

    replica_groups = [list(range(c.NCORES))]

    with TileContext(nc) as tc:
        with (
            tc.tile_pool(name="persist", bufs=1) as pp,
            tc.tile_pool(name="gidx", bufs=8) as gip,
            tc.tile_pool(name="work", bufs=3) as wp,
        ):
            with tc.high_priority():
                nc.gpsimd.load_library(library_config.mlp)

            # persistent SBUF
            a_d1 = pp.tile([128, NB * 4], F32)       # a_d layer1 (node-major)
            a_d2 = pp.tile([128, NB], F32)           # a_d layer2
            comp1 = pp.tile([128, NB * c.ROW1C], F32)  # compact xps1 slice
            outcat = pp.tile([128, NB * 36], F32)    # L1: [S(4) | out_un(32)]
            hcat = pp.tile([128, NB * 32], F32)      # h after elu
            comp2 = pp.tile([128, NB * c.ROW2C], F32)
            out2cat = pp.tile([128, NB * 9], F32)    # L2: [S2(1) | out2_un(8)]
            b1t = pp.tile([128, 32], F32)
            b2t = pp.tile([128, 8], F32)
            pnt = pp.tile([128, NB], F32)
            pot = pp.tile([128, NB], F32)
            nit = pp.tile([1, NB * cfg.GROUPS], mybir.dt.int32)
            nc.sync.dma_start(b1t[:], b1r[:])
            nc.sync.dma_start(b2t[:], b2r[:])
            nc.sync.dma_start(pnt[:], padneg[:])
            nc.sync.dma_start(pot[:], padone[:])
            nc.sync.dma_start(nit[:], nidx[:])

            # ---------------- Phase A: xps1 = [x @ W1ext] ----------------
            w1sb = pp.tile([128, 4, 40], F32)
            nc.sync.dma_start(w1sb[:], w1e[:].rearrange("(k p) n -> p k n", p=128))
            ident = pp.tile([128, 128], F32)
            from concourse.masks import make_identity
            make_identity(nc, ident[:])

            NT = 512  # nodes per matmul tile
            mp_cm = tc.tile_pool(name="mm", bufs=3)
            mp = mp_cm.__enter__()
            psp_cm = tc.tile_pool(name="mmpa", bufs=2, space="PSUM"); psp = psp_cm.__enter__()
            for t0 in range(0, c.NPAD, NT):
                nt = min(NT, c.NPAD - t0)
                xtile = mp.tile([128, 4, NT], F32, tag="xt")
                nc.sync.dma_start(xtile[:, :, :nt],
                                  xT[:, t0:t0 + nt].rearrange("(k p) n -> p k n", p=128))
                ps = psp.tile([40, NT], F32, tag="mm1")
                for k in range(4):
                    nc.tensor.matmul(ps[:, :nt], w1sb[:, k, :], xtile[:, k, :nt],
                                     start=(k == 0), stop=(k == 3))
                xpsT = mp.tile([40, NT], F32, tag="xpsT")
                nc.scalar.copy(xpsT[:, :nt], ps[:, :nt])
                # transpose per 128-node chunk -> node-major
                for s0 in range(0, nt, 128):
                    b = (t0 + s0) // 128
                    pst = psp.tile([128, 40], F32, tag="tr1")
                    nc.tensor.transpose(pst[:], xpsT[:, s0:s0 + 128], ident[:40, :40])
                    nm = wp.tile([128, 40], F32, tag="nm")
                    nc.scalar.copy(nm[:], pst[:])
                    # nm layout = [a_s(4) | xp(32) | a_d(4)] (W1ext order)
                    # pad nodes get a_s = -1e30 (additive mask input)
                    nc.vector.tensor_add(
                        nm[:, 0:4], nm[:, 0:4],
                        pnt[:, b:b + 1].broadcast_to([128, 4]))
                    # a_d -> resident; compact row [a_s | xp] in one copy
                    nc.vector.tensor_copy(a_d1[:, b * 4:(b + 1) * 4], nm[:, 36:40])
                    nc.vector.tensor_copy(
                        comp1[:, b * c.ROW1C:(b + 1) * c.ROW1C], nm[:, 0:36])

            psp_cm.__exit__(None, None, None)
            mp_cm.__exit__(None, None, None)
            # write compact slice (partition-major) + allgather + expand
            nc.sync.dma_start(
                tc1_in[:].rearrange("(p w) -> p w", p=128), comp1[:])
            nc.gpsimd.collective_compute(
                "AllGather", ALU.bypass,
                ins=[tc1_in[:]], outs=[tc1_full[:]],
                replica_groups=replica_groups,
            )
            _expand_table(nc, tc, wp, cfg, tc1_full, tbl1, c.ROW1C)

            # ---------------- L1 edge phase (epilogue interleaved) --------
            gp_cm = tc.tile_pool(name="gat", bufs=5)
            gp = gp_cm.__enter__()
            tp_cm = tc.tile_pool(name="tmp", bufs=2)
            tpool = tp_cm.__enter__()
            w2sb = pp.tile([32, 12], F32)
            nc.sync.dma_start(w2sb[:], w2e[:])
            psp_cm = tc.tile_pool(name="mmpb", bufs=2, space="PSUM")
            psp = psp_cm.__enter__()

            def epi1(b):
                """h = elu(out/S + b1); xps2 = h @ W2ext; stash compact row."""
                S = outcat[:, b * 36:b * 36 + 4]
                nc.vector.tensor_add(
                    S, S, pot[:, b:b + 1].broadcast_to([128, 4]))
                ou = outcat[:, b * 36 + 4:(b + 1) * 36]
                r = wp.tile([128, 4], F32, tag="r1")
                nc.vector.reciprocal(r[:], S)
                z = wp.tile([128, 32], F32, tag="z")
                nc.vector.tensor_tensor(
                    out=z[:].rearrange("p (h c) -> p h c", h=4),
                    in0=ou.rearrange("p (h c) -> p h c", h=4),
                    in1=r[:, :, None].broadcast_to([128, 4, 8]),
                    op=ALU.mult)
                nc.vector.tensor_add(z[:], z[:], b1t[:])
                # elu: h = max(z,0) + exp(min(z,0)) - 1
                mneg = wp.tile([128, 32], F32, tag="mneg")
                nc.vector.tensor_scalar(out=mneg[:], in0=z[:], scalar1=0.0,
                                        scalar2=None, op0=ALU.min)
                q = wp.tile([128, 32], F32, tag="q")
                nc.scalar.activation(q[:], mneg[:], ACTF.Exp)
                h = hcat[:, b * 32:(b + 1) * 32]
                nc.vector.tensor_scalar(out=h, in0=z[:], scalar1=0.0,
                                        scalar2=None, op0=ALU.max)
                nc.vector.tensor_add(h, h, q[:])
                nc.vector.tensor_scalar_add(h, h, -1.0)
                # xps2 = h @ W2ext : transpose h -> [32, 128]
                psh = psp.tile([32, 128], F32, tag="trh")
                nc.tensor.transpose(psh[:], h, ident[:])
                hT = wp.tile([32, 128], F32, tag="hT")
                nc.scalar.copy(hT[:], psh[:])
                ps2 = psp.tile([12, 128], F32, tag="mm2")
                nc.tensor.matmul(ps2[:], w2sb[:], hT[:], start=True, stop=True)
                x2T = wp.tile([12, 128], F32, tag="x2T")
                nc.scalar.copy(x2T[:], ps2[:])
                ps3 = psp.tile([128, 12], F32, tag="tr2")
                nc.tensor.transpose(ps3[:], x2T[:], ident[:12, :12])
                nm2 = wp.tile([128, 12], F32, tag="nm2")
                nc.scalar.copy(nm2[:], ps3[:])
                # nm2 layout = [a_s2 | xp2(8) | a_d2 | pad] (W2ext order)
                nc.vector.tensor_add(nm2[:, 0:1], nm2[:, 0:1], pnt[:, b:b + 1])
                nc.vector.tensor_copy(a_d2[:, b:b + 1], nm2[:, 9:10])
                nc.vector.tensor_copy(comp2[:, b * 9:(b + 1) * 9], nm2[:, 0:9])

            _edge_layer(nc, tc, cfg, plan, gp, gip, wp, tpool, idxt, tbl1,
                        a_d1, outcat, layer=1, nit=nit, epi_cb=epi1)

            psp_cm.__exit__(None, None, None)
            nc.sync.dma_start(
                tc2_in[:].rearrange("(p w) -> p w", p=128), comp2[:])
            nc.gpsimd.collective_compute(
                "AllGather", ALU.bypass,
                ins=[tc2_in[:]], outs=[tc2_full[:]],
                replica_groups=replica_groups,
            )
            _expand_table(nc, tc, wp, cfg, tc2_full, tbl2, c.ROW2C)

            # ---------------- L2 edge phase (final epilogue interleaved) --
            def epi2(b):
                S2 = out2cat[:, b * 9:b * 9 + 1]
                nc.vector.tensor_add(S2, S2, pot[:, b:b + 1])
                ou2 = out2cat[:, b * 9 + 1:(b + 1) * 9]
                r2 = wp.tile([128, 1], F32, tag="r2")
                nc.vector.reciprocal(r2[:], S2)
                fo = wp.tile([128, 8], F32, tag="fo")
                nc.vector.tensor_scalar(out=fo[:], in0=ou2, scalar1=r2[:],
                                        scalar2=None, op0=ALU.mult)
                nc.vector.tensor_add(fo[:], fo[:], b2t[:])
                nc.sync.dma_start(y[:, b * 8:(b + 1) * 8], fo[:])

            _edge_layer(nc, tc, cfg, plan, gp, gip, wp, tpool, idxt, tbl2,
                        a_d2, out2cat, layer=2, nit=nit, epi_cb=epi2)
            tp_cm.__exit__(None, None, None)
            gp_cm.__exit__(None, None, None)

    nc.finalize()
    return nc



def _dma_gather_raw(gps, out_ap, in_ap, idxs_ap, num_idxs, elem_size,
                    elem_step, queue_num, num_idxs_reg=None):
    """bass.BassGpSimd.dma_gather with the elem_size%256 assert relaxed to %4
    (the Q7 ucode handles arbitrary element lengths; verified on HW).

    num_idxs_reg: optional dynamic count (<= num_idxs). Must equal the
    post-trim count (trailing negative idxs) so the decode-side ring
    bookkeeping stays in lockstep with the Q7 descriptor pushes."""
    from concourse import ap_utils
    from concourse.bass import MemorySpace
    import concourse.mybir as mb

    assert idxs_ap.dtype == I16
    assert in_ap.dtype == out_ap.dtype
    elem_size_bytes = elem_size * mb.dt.size(in_ap.dtype)
    assert elem_size_bytes > 0 and elem_size_bytes % 4 == 0
    assert in_ap.space == MemorySpace.DRAM
    assert idxs_ap.space == MemorySpace.SBUF
    assert out_ap.space == MemorySpace.SBUF
    assert ap_utils.ap_is_contiguous(out_ap.ap[1:])
    assert ap_utils.ap_is_contiguous(idxs_ap.ap[1:])
    assert in_ap.ap[-1][1] == out_ap.ap[-1][1] == elem_size
    assert out_ap.ap[0][1] * out_ap.ap[1][1] == ((num_idxs + 127) // 128) * 128
    assert in_ap.ap[0][0] == elem_step
    stride_bytes = elem_step * mb.dt.size(in_ap.dtype)
    assert stride_bytes % 256 == 0
    stride_bytes_256 = stride_bytes // 256
    assert stride_bytes_256 < 256

    _in_ap = gps.lower_ap_dma(in_ap, for_custom_bir_dma=True)
    _idxs_ap = gps.lower_ap(idxs_ap)
    _out_ap = gps.lower_ap(out_ap)
    if num_idxs_reg is None:
        num_idxs_reg = num_idxs
    return gps.add_instruction(
        mb.InstDMAGatherAnt(
            name=gps.bass.get_next_instruction_name(),
            ins=[*_in_ap, _idxs_ap,
                 gps.lower_val_access(gps.to_reg(num_idxs_reg))],
            outs=[_out_ap],
            transpose=False,
            num_idxs=num_idxs,
            elem_size=elem_size,
            stride_bytes_256=stride_bytes_256,
            gen_mode=0,
            single_packet=False,
            queue_num=queue_num,
        )
    )


def _expand_table(nc, tc, wp_unused, cfg, compact_dram, padded_dram, roww):
    """Expand compact rows [TROWS, roww] (flat) to 256B rows [TROWS, 64].
    Group-ordered: each int16-addressable table quarter is expanded in
    sequence (full 128-partition width within the quarter), so group-g
    gathers can begin as soon as quarter g is written."""
    c = cfg
    GR = c.GROUP_ROWS              # rows per group (25088)
    assert GR % 128 == 0
    rpp = GR // 128                # rows per partition within a group
    CH = 4
    while rpp % CH != 0:
        CH -= 1
    rch = rpp // CH
    ep_cm = tc.tile_pool(name=f"exp{roww}", bufs=2)
    ep = ep_cm.__enter__()
    for g in range(c.GROUPS):
        srcg = compact_dram[g * GR * roww:(g + 1) * GR * roww].rearrange(
            "(p r w) -> p r w", p=128, w=roww)
        dstg = padded_dram[g * GR:(g + 1) * GR, :].rearrange(
            "(p r) w -> p r w", p=128)
        for ch in range(CH):
            ct = ep.tile([128, rch, roww], F32, tag="exp_in")
            nc.sync.dma_start(ct[:], srcg[:, ch * rch:(ch + 1) * rch, :])
            # write only the roww words the gather reads (elem_size < stride);
            # words [roww, 64) of each 256B row stay garbage, never fetched
            nc.sync.dma_start(
                dstg[:, ch * rch:(ch + 1) * rch, :roww], ct[:])
    ep_cm.__exit__(None, None, None)


def _edge_layer(nc, tc, cfg, plan, gp, gip, wp, tpool, idxt, tbl, a_d, outcat,
                layer, nit=None, epi_cb=None, lag=2):
    """Edge phase: per (block, group) gather + attention + aggregation."""
    c = cfg
    H = c.H1 if layer == 1 else c.H2        # heads
    CC = c.C1 if layer == 1 else c.C2       # channels/head
    aw = 4 if layer == 1 else 1             # a_s words at row start
    xw = H * CC                             # xp words
    GP_BUFS = 5
    idx_off = 0
    Dmax = int(plan.D.sum(1).max())
    nregs = None
    if USE_TRIM and nit is not None:
        nregs = [nc.gpsimd.alloc_register(f"nidx_l{layer}_q{g}")
                 for g in range(c.GROUPS)]
    for b in range(c.NBLK):
        Dt = int(plan.D[b].sum())           # total slots/partition this block
        RW = 4 + xw                        # gathered words per row
        Gf = gp.tile([128, Dmax, RW], F32, tag=f"G{layer}")
        G = Gf[:, :Dt, :]
        if b < GP_BUFS:
            # first rotation: clear stale SBUF (NaN-safe: pv=0 * garbage)
            nc.vector.memset(Gf[:], 0.0)
        # one idx DMA per block (group segments are adjacent in idxt)
        itf = gip.tile([128, 8 * Dmax], I16, tag="it")
        itb = itf[:, :8 * Dt]
        nc.sync.dma_start(itb[:], idxt[:, idx_off:idx_off + 8 * Dt])
        idx_off += 8 * Dt
        off = 0
        for g in range(c.GROUPS):
            Dg = int(plan.D[b, g])
            Dmin_g = int(plan.Dmin[b, g])
            if Dmin_g < Dg:
                # columns this core may trim: force p = 0 via a_s = -inf
                nc.vector.memset(G[:, off + Dmin_g:off + Dg, 0:aw], NEG_BIG)
            nsl = 128 * Dg
            nreg = None
            if nregs is not None:
                k = b * c.GROUPS + g
                nreg = nregs[g]
                nc.gpsimd.reg_load(nreg, nit[0:1, k:k + 1])
            _dma_gather_raw(
                nc.gpsimd,
                G[:, off:off + Dg, :],
                tbl[g * c.GROUP_ROWS:(g + 1) * c.GROUP_ROWS, :RW],
                itb[:, 8 * off:8 * (off + Dg)], nsl, RW, c.ROWP,
                queue_num=g % 4, num_idxs_reg=nreg,
            )
            off += Dg
        Hm = cfg.H1
        if USE_ACTFUSE:
            # lr = leaky_relu(a_s + a_d) fused on the ACT engine
            lrf = wp.tile([128, Hm, Dmax], F32, tag="lr")
            lr = lrf[:, :H, :Dt]
            for h in range(H):
                nc.scalar.activation(
                    lr[:, h, :], G[:, :, h], ACTF.Prelu,
                    bias=a_d[:, b * H + h:b * H + h + 1], scale=1.0,
                    alpha=c.NEG_SLOPE)
            # p = exp(lr); S = sum_j p via the ACT accumulator
            pvf = wp.tile([128, Hm, Dmax], F32, tag="p")
            pv = pvf[:, :H, :Dt]
            for h in range(H):
                nc.scalar.activation(
                    pv[:, h, :], lr[:, h, :], ACTF.Exp,
                    accum_out=outcat[:, b * (H + xw) + h:b * (H + xw) + h + 1])
        else:
            epf = wp.tile([128, Hm, Dmax], F32, tag="e")
            ep = epf[:, :H, :Dt]
            for h in range(H):
                nc.scalar.activation(
                    ep[:, h, :], G[:, :, h], ACTF.Identity,
                    bias=a_d[:, b * H + h:b * H + h + 1], scale=1.0)
            lrf = wp.tile([128, Hm, Dmax], F32, tag="lr")
            lr = lrf[:, :H, :Dt]
            nc.vector.tensor_scalar(out=lr[:], in0=ep[:], scalar1=0.0,
                                    scalar2=c.NEG_SLOPE, op0=ALU.min,
                                    op1=ALU.mult)
            pposf = wp.tile([128, Hm, Dmax], F32, tag="ppos")
            ppos = pposf[:, :H, :Dt]
            nc.vector.tensor_scalar(out=ppos[:], in0=ep[:], scalar1=0.0,
                                    scalar2=None, op0=ALU.max)
            nc.vector.tensor_add(lr[:], lr[:], ppos[:])
            pvf = wp.tile([128, Hm, Dmax], F32, tag="p")
            pv = pvf[:, :H, :Dt]
            nc.scalar.activation(pv[:], lr[:], ACTF.Exp)
            nc.vector.tensor_reduce(
                out=outcat[:, b * (H + xw):b * (H + xw) + H],
                in_=pv[:], op=ALU.add, axis=AX)
        # msg = p (bcast over CC) * xp ; out_un = sum_j msg
        tmpf = tpool.tile([128, c.H1 * c.C1, Dmax], F32, tag="tmp")
        tmp = tmpf[:, :H * CC, :Dt]
        if H > 1:
            nc.vector.tensor_tensor(
                out=tmp[:].rearrange("p (h c) d -> p h c d", h=H),
                in0=pv[:, :, None, :].broadcast_to([128, H, CC, Dt]),
                in1=G[:, :, aw:aw + xw].rearrange("p d (h c) -> p h c d", h=H),
                op=ALU.mult)
        else:
            # 3D form: a size-1 head dim lowers to a pathologically slow
            # DVE instruction
            nc.vector.tensor_tensor(
                out=tmp[:],
                in0=pv[:, 0, None, :].broadcast_to([128, CC, Dt]),
                in1=G[:, :, aw:aw + xw].rearrange("p d c -> p c d"),
                op=ALU.mult)
        nc.vector.tensor_reduce(
            out=outcat[:, b * (H + xw) + H:(b + 1) * (H + xw)],
            in_=tmp[:], op=ALU.add, axis=AX)
        # interleave the per-block epilogue under the (desc-gen-bound)
        # edge phase so it rides in the engine-queue shadow
        if epi_cb is not None and b - lag >= 0:
            epi_cb(b - lag)
    if epi_cb is not None:
        for bb in range(max(c.NBLK - lag, 0), c.NBLK):
            epi_cb(bb)


# ----------------------------------------------------------------------------
# Host wrapper
# ----------------------------------------------------------------------------
def _build_w1ext(W1, att_src1, att_dst1):
    # [W1@As | W1 | W1@Ad]: As[j, h] = att_src1[h, j%C] if j//C==h
    H, C = att_src1.shape
    As = np.zeros((H * C, H), np.float32)
    Ad = np.zeros((H * C, H), np.float32)
    for h in range(H):
        As[h * C:(h + 1) * C, h] = att_src1[h]
        Ad[h * C:(h + 1) * C, h] = att_dst1[h]
    return np.concatenate([W1 @ As, W1, W1 @ Ad], axis=1).astype(np.float32)


def _build_w2ext(W2, att_src2, att_dst2):
    H, C = att_src2.shape
    As = att_src2.reshape(C, 1).astype(np.float32)
    Ad = att_dst2.reshape(C, 1).astype(np.float32)
    out = np.concatenate([W2 @ As, W2, W2 @ Ad, np.zeros((32, 2), np.float32)],
                         axis=1)
    return out.astype(np.float32)


def _pad_masks(cfg, node_at):
    """[128, NB] additive masks: NEG_BIG / 1.0 on pad (rank) positions."""
    c = cfg
    is_pad = (node_at < 0).reshape(c.NBLK, 128).T  # [128, NB]
    neg = np.where(is_pad, np.float32(NEG_BIG), np.float32(0.0))
    one = np.where(is_pad, np.float32(1.0), np.float32(0.0))
    return np.ascontiguousarray(neg), np.ascontiguousarray(one)


LAST_EXEC_NS = None


def kernel(x, edge_index, W1, att_src1, att_dst1, b1, W2, att_src2, att_dst2,
           b2):
    cfg = Cfg(N=x.shape[0], E=edge_index.shape[1], F_IN=x.shape[1])
    plan = Plan(cfg, np.asarray(edge_index))
    nc = build_kernel(cfg, plan)

    x = np.asarray(x, dtype=np.float32)
    w1e = _build_w1ext(np.asarray(W1), np.asarray(att_src1), np.asarray(att_dst1))
    w2e = _build_w2ext(np.asarray(W2), np.asarray(att_src2), np.asarray(att_dst2))
    b1r = np.tile(np.asarray(b1, np.float32)[None, :], (128, 1))
    b2r = np.tile(np.asarray(b2, np.float32)[None, :], (128, 1))

    in_maps = []
    for ci in range(cfg.NCORES):
        na = plan.node_at[ci]
        xs = np.zeros((cfg.NPAD, cfg.F_IN), np.float32)
        realm = na >= 0
        xs[realm] = x[ci * cfg.NPC:(ci + 1) * cfg.NPC][na[realm]]
        neg, one = _pad_masks(cfg, na)
        nidx = (128 * plan.Dk[ci].reshape(1, -1)).astype(np.int32)
        if not USE_TRIM:
            nidx = (128 * np.broadcast_to(
                plan.D.reshape(1, -1), nidx.shape)).astype(np.int32)
        in_maps.append({
            "xT": np.ascontiguousarray(xs.T),
            "w1e": w1e, "w2e": w2e, "b1r": b1r, "b2r": b2r,
            "padneg": neg, "padone": one,
            "idxt": plan.idx_planes[ci],
            "nidx": nidx,
        })

    global LAST_EXEC_NS
    want_trace = False
    try:
        from antenv.axon_hooks import get_axon_ntff_profile_hook
        want_trace = get_axon_ntff_profile_hook() is not None
    except ImportError:
        pass
    res = run_bass_kernel_spmd(nc, in_maps, core_ids=list(range(cfg.NCORES)),
                               trace=want_trace)
    LAST_EXEC_NS = res.exec_time_ns

    out = np.empty((cfg.N, 8), np.float32)
    for ci in range(cfg.NCORES):
        yv = res.results[ci]["y"].reshape(128, cfg.NBLK, 8)
        na = plan.node_at[ci]
        ranks = np.arange(cfg.NPAD)
        realm = na >= 0
        out[ci * cfg.NPC + na[realm]] = yv[ranks[realm] % 128,
                                           ranks[realm] // 128, :]
    return out


if __name__ == "__main__":
    pass


# revision 35
# speedup vs baseline: 1.0927x; 1.0927x over previous
"""2-layer GAT (PyG-style, eval mode) on 8 Trainium2 NeuronCores via Bass/Tile.

Architecture (dst-sharded, destination-bucketed, dma_gather based):
  - Destination nodes are sharded across 8 cores (12500 each, padded 12544).
  - Per core, dst nodes are packed into 98 blocks of 128 (partition dim) by
    recursive coordinate bisection on their per-group in-edge counts, so the
    per-(block, group) max-over-partition slot counts stay near the mean
    (slot padding ~1.29x vs 2.05x for plain degree sort).
  - Per-edge source rows ([a_s | xp], 256B-strided table) are fetched with
    InstDMAGatherAnt over 4 SWDGE queues (one per int16-addressable table
    quarter). Each core's index planes carry trailing -1 columns per
    (block, group); the Q7 ucode trims trailing negatives, so descriptor
    generation (the kernel's bottleneck) runs at each core's own padding
    even though instruction shapes are baked to the cross-core max. The
    cross-core slack columns are masked by memsetting their a_s lanes to
    -1e30 before the gather lands (=> p = 0).
  - e = a_s + a_d, leaky-relu, and exp all run on the Activation engine
    (Lrelu with bias/alpha, then Exp with accum_out producing the softmax
    denominator), keeping DVE for just the p*xp multiply + reduce.
  - Segment softmax folds into: p = exp(leaky_relu(a_s + a_d)); S = sum_j p;
    out = (sum_j p * xp) / S  (no segment-max: attention logits are O(1), so
    exp cannot overflow; this matches the reference up to fp rounding).
  - Node-feature tables are exchanged compactly with AllGather and expanded
    to the 256B-aligned gather layout on device with fat DMAs + DVE copies.
"""

import os
import sys

sys.path.insert(0, "/opt/trn_rl_repo")

import numpy as np

# bisect switches (default on; set GAT_NO_TRIM=1 / GAT_NO_ACTFUSE=1 to disable)
USE_TRIM = os.environ.get("GAT_NO_TRIM", "") != "1"
USE_ACTFUSE = os.environ.get("GAT_NO_ACTFUSE", "") != "1"

import concourse.bass as bass
import concourse.bacc as bacc
import concourse.mybir as mybir
from concourse.tile import TileContext
from concourse import library_config
from concourse.bass_utils import run_bass_kernel_spmd

F32 = mybir.dt.float32
I16 = mybir.dt.int16

AX = mybir.AxisListType.X
ALU = mybir.AluOpType
ACTF = mybir.ActivationFunctionType

NEG_BIG = -1.0e30


class Cfg:
    def __init__(self, N=100000, E=3200000, F_IN=512, ncores=8):
        self.N = N
        self.E = E
        self.F_IN = F_IN
        self.H1, self.C1 = 4, 8
        self.H2, self.C2 = 1, 8
        self.NEG_SLOPE = 0.2
        self.NCORES = ncores
        assert N % ncores == 0
        self.NPC = N // ncores                      # real nodes per core
        self.NPAD = ((self.NPC + 127) // 128) * 128  # padded
        self.NBLK = self.NPAD // 128
        self.GROUPS = 4
        assert (self.NPAD * ncores) % self.GROUPS == 0
        self.GROUP_ROWS = self.NPAD * ncores // self.GROUPS  # table rows/group
        assert self.GROUP_ROWS <= 32767
        # compact row layouts (fp32 words)
        self.ROW1C = 4 + self.H1 * self.C1   # 36: [a_s1(4) | xp1(32)]
        self.ROW2C = 1 + self.C2             # 9:  [a_s2(1) | xp2(8)]
        self.ROWP = 64                       # padded row: 256B


# ----------------------------------------------------------------------------
# Host-side preprocessing: sharding, packing, slot/idx construction
# ----------------------------------------------------------------------------
def _rcb_order(cnt, npad):
    """Recursive coordinate bisection on per-group counts; returns
    rank -> row index into cnt (rows >= len(cnt) are zero-profile pads)."""
    npc = cnt.shape[0]
    cntp = np.zeros((npad, cnt.shape[1]), cnt.dtype)
    cntp[:npc] = cnt
    out = []

    def bisect(ids):
        n = len(ids)
        if n <= 128:
            out.append(ids)
            return
        vals = cntp[ids]
        g = int(np.argmax(vals.std(0)))
        srt = np.argsort(vals[:, g], kind="stable")
        h = max(128, ((n // 2) // 128) * 128)
        bisect(ids[srt[:h]])
        bisect(ids[srt[h:]])

    bisect(np.arange(npad))
    return np.concatenate(out)


def _refine(node_at, cnt, nblk, deadline=3.0):
    """Local search: swap per-(block,group) argmax nodes into blocks where
    they fit under the maxima. Mutates and returns node_at ([npad] rank->row,
    pads included as zero-profile rows)."""
    import time

    t0 = time.time()
    npad = node_at.size
    prof = np.zeros((npad, cnt.shape[1]), cnt.dtype)
    real = node_at < cnt.shape[0]
    prof[real] = cnt[node_at[real]]
    P = prof.reshape(nblk, 128, -1)
    NA = node_at.reshape(nblk, 128)
    for _ in range(40):
        M = P.max(axis=1)
        swaps = 0
        for b in range(nblk):
            for g in range(P.shape[2]):
                col = P[b, :, g]
                mx = col.max()
                lower = col[col < mx]
                if lower.size == 0:
                    continue
                second = lower.max()
                if mx - second <= 0:
                    continue
                attain = np.where(col == mx)[0]
                if len(attain) > 2:
                    continue
                p = int(attain[0])
                cu = P[b, p].copy()
                fit_b = (cu[None, :] <= M).all(1)
                fit_b[b] = False
                for b2 in np.where(fit_b)[0]:
                    cand = (P[b2, :, g] <= second) & (
                        P[b2] <= M[b][None, :]).all(1)
                    if cand.any():
                        p2 = int(np.argmax(cand))
                        P[b, p], P[b2, p2] = P[b2, p2].copy(), P[b, p].copy()
                        NA[b, p], NA[b2, p2] = NA[b2, p2], NA[b, p]
                        M[b] = P[b].max(0)
                        M[b2] = P[b2].max(0)
                        swaps += 1
                        break
        if swaps == 0 or time.time() - t0 > deadline:
            break
    return node_at


class Plan:
    """Host plan: per-core node placement (rank -> node), slot structure
    (global shapes + per-core trailing trim) and gather index planes."""

    def __init__(self, cfg: Cfg, edge_index: np.ndarray):
        c = cfg
        src0 = edge_index[0].astype(np.int64)
        dst0 = edge_index[1].astype(np.int64)
        loop = np.arange(c.N, dtype=np.int64)
        src = np.concatenate([src0, loop])
        dst = np.concatenate([dst0, loop])

        grp_of_src_node = (src // c.NPC) // 2 if c.NCORES == 8 else \
            np.minimum((src // c.NPC) * c.GROUPS // c.NCORES, c.GROUPS - 1)

        core_of = dst // c.NPC
        # ---- per-core packing ----
        # node_at[ci][r] = local node at rank r, or -1 (pad)
        self.node_at = []
        per_core = []
        cnts = []
        for ci in range(c.NCORES):
            m = core_of == ci
            s_c, d_c = src[m], dst[m] - ci * c.NPC
            g_c = grp_of_src_node[m]
            cnt = np.zeros((c.NPC, c.GROUPS), np.int64)
            np.add.at(cnt, (d_c, g_c), 1)
            order = _rcb_order(cnt, c.NPAD)
            order = _refine(order, cnt, c.NBLK, deadline=6.0)
            na = np.where(order < c.NPC, order, -1)
            self.node_at.append(na)
            per_core.append((s_c, d_c))
            cnts.append(cnt)

        # global "rank" -> table row of any node (as a source):
        # node n (core k, local l) at rank r -> row k*NPAD + NBLK*(r%128)+r//128
        self.rank_of_global = np.empty(c.N, dtype=np.int64)
        self.pad_rows = []   # per core: table rows of pad ranks
        for ci in range(c.NCORES):
            na = self.node_at[ci]
            ranks = np.arange(c.NPAD)
            rc = c.NBLK * (ranks % 128) + ranks // 128
            realm = na >= 0
            self.rank_of_global[ci * c.NPC + na[realm]] = ci * c.NPAD + rc[realm]
            self.pad_rows.append(ci * c.NPAD + rc[~realm])

        tbl_row = self.rank_of_global
        # ---- per-core per-(block, group) counts ----
        self.Dk = np.zeros((c.NCORES, c.NBLK, c.GROUPS), np.int64)
        self.core_edges = []
        for ci in range(c.NCORES):
            s_c, d_c = per_core[ci]
            na = self.node_at[ci]
            rank_of = np.empty(c.NPC, np.int64)
            ranks = np.arange(c.NPAD)
            realm = na >= 0
            rank_of[na[realm]] = ranks[realm]
            r_c = rank_of[d_c]
            b = r_c // 128
            p = r_c % 128
            g = tbl_row[s_c] // c.GROUP_ROWS
            counts = np.zeros((c.NBLK, c.GROUPS, 128), np.int64)
            np.add.at(counts, (b, g, p), 1)
            self.Dk[ci] = counts.max(axis=2)
            self.core_edges.append((s_c, r_c, b, p, g))
        self.D = np.maximum(self.Dk.max(axis=0), 1)   # global shapes
        self.Dmin = self.Dk.min(axis=0)               # memset slack floor
        self.slots_bg = 128 * self.D
        self.tot_slots = int(self.slots_bg.sum())

        # dummy row per group: a pad row of core 2g (core-major table layout
        # puts cores 2g,2g+1 inside group g). Pad rows get a_s=-1e30 via the
        # per-block pad mask, so p = 0 for slots pointing at them.
        self.dummy_local = np.zeros(c.GROUPS, dtype=np.int64)
        for g in range(c.GROUPS):
            k = (g * c.GROUP_ROWS) // c.NPAD
            cand = [r for r in self.pad_rows[k]
                    if g * c.GROUP_ROWS <= r < (g + 1) * c.GROUP_ROWS]
            if not cand:
                for kk in range(c.NCORES):
                    cand = [r for r in self.pad_rows[kk]
                            if g * c.GROUP_ROWS <= r < (g + 1) * c.GROUP_ROWS]
                    if cand:
                        break
            assert cand, f"no pad row available for group {g}"
            self.dummy_local[g] = cand[0] - g * c.GROUP_ROWS

        # gather idx planes per core: concat over (b, g) of wrapped
        # [128, 8*D] int16 planes; trailing columns beyond this core's own
        # Dk are -1 (trimmed by the gather ucode => no descriptors).
        self.idx_planes = []
        for ci in range(c.NCORES):
            s_c, r_c, b, p, g = self.core_edges[ci]
            segs = []
            for bb in range(c.NBLK):
                for gg in range(c.GROUPS):
                    Dn = int(self.D[bb, gg])
                    Dkn = int(self.Dk[ci, bb, gg])
                    nslots = 128 * Dn
                    sel = (b == bb) & (g == gg)
                    pp = p[sel]
                    loc = tbl_row[s_c[sel]] - gg * c.GROUP_ROWS
                    ordr = np.argsort(pp, kind="stable")
                    pp_s = pp[ordr]
                    loc_s = loc[ordr]
                    jj = np.arange(pp_s.size) - np.searchsorted(pp_s, pp_s)
                    slot = jj * 128 + pp_s
                    arr = np.full(nslots, self.dummy_local[gg], dtype=np.int16)
                    arr[slot] = loc_s.astype(np.int16)
                    if USE_TRIM:
                        arr[128 * Dkn:] = -1
                    segs.append(arr.reshape(-1, 16).T)  # [16, nslots/16]
            wrapped = np.concatenate(segs, axis=1)
            plane = np.tile(wrapped, (8, 1)).astype(np.int16)
            self.idx_planes.append(plane)
        self.idx_cols = self.idx_planes[0].shape[1]


# ----------------------------------------------------------------------------
# Device kernel builder (one program, SPMD on 8 cores)
# ----------------------------------------------------------------------------
def build_kernel(cfg: Cfg, plan: Plan):
    c = cfg
    NB = c.NBLK
    TROWS = c.NPAD * c.NCORES          # padded table rows (100352)
    nc = bacc.Bacc(num_swdge_queues=4, num_devices=c.NCORES)

    # ---- inputs ----
    xT = nc.dram_tensor("xT", [c.F_IN, c.NPAD], F32, kind="ExternalInput")
    w1e = nc.dram_tensor("w1e", [c.F_IN, 40], F32, kind="ExternalInput")
    w2e = nc.dram_tensor("w2e", [32, 12], F32, kind="ExternalInput")
    b1r = nc.dram_tensor("b1r", [128, 32], F32, kind="ExternalInput")
    b2r = nc.dram_tensor("b2r", [128, 8], F32, kind="ExternalInput")
    padneg = nc.dram_tensor("padneg", [128, NB], F32, kind="ExternalInput")
    padone = nc.dram_tensor("padone", [128, NB], F32, kind="ExternalInput")
    idxt = nc.dram_tensor("idxt", [128, plan.idx_cols], I16, kind="ExternalInput")
    nidx = nc.dram_tensor("nidx", [1, NB * cfg.GROUPS], mybir.dt.int32,
                          kind="ExternalInput")
    y = nc.dram_tensor("y", [128, NB * 8], F32, kind="ExternalOutput")

    # ---- internal DRAM ----
    tc1_in = nc.dram_tensor("tc1_in", [128 * NB * c.ROW1C], F32, kind="Internal")
    tc1_full = nc.dram_tensor("tc1_full", [TROWS * c.ROW1C], F32,
                              kind="Internal", addr_space="Shared")
    tbl1 = nc.dram_tensor("tbl1", [TROWS, c.ROWP], F32, kind="Internal")
    tc2_in = nc.dram_tensor("tc2_in", [128 * NB * c.ROW2C], F32, kind="Internal")
    tc2_full = nc.dram_tensor("tc2_full", [TROWS * c.ROW2C], F32,
                              kind="Internal", addr_space="Shared")
    tbl2 = nc.dram_tensor("tbl2", [TROWS, c.ROWP], F32, kind="Internal")

    replica_groups = [list(range(c.NCORES))]

    with TileContext(nc) as tc:
        with (
            tc.tile_pool(name="persist", bufs=1) as pp,
            tc.tile_pool(name="gidx", bufs=8) as gip,
            tc.tile_pool(name="work", bufs=3) as wp,
        ):
            with tc.high_priority():
                nc.gpsimd.load_library(library_config.mlp)

            # persistent SBUF
            a_d1 = pp.tile([128, NB * 4], F32)       # a_d layer1 (node-major)
            a_d2 = pp.tile([128, NB], F32)           # a_d layer2
            comp1 = pp.tile([128, NB * c.ROW1C], F32)  # compact xps1 slice
            outcat = pp.tile([128, NB * 36], F32)    # L1: [S(4) | out_un(32)]
            hcat = pp.tile([128, NB * 32], F32)      # h after elu
            comp2 = pp.tile([128, NB * c.ROW2C], F32)
            out2cat = pp.tile([128, NB * 9], F32)    # L2: [S2(1) | out2_un(8)]
            b1t = pp.tile([128, 32], F32)
            b2t = pp.tile([128, 8], F32)
            pnt = pp.tile([128, NB], F32)
            pot = pp.tile([128, NB], F32)
            nit = pp.tile([1, NB * cfg.GROUPS], mybir.dt.int32)
            nc.sync.dma_start(b1t[:], b1r[:])
            nc.sync.dma_start(b2t[:], b2r[:])
            nc.sync.dma_start(pnt[:], padneg[:])
            nc.sync.dma_start(pot[:], padone[:])
            nc.sync.dma_start(nit[:], nidx[:])

            # ---------------- Phase A: xps1 = [x @ W1ext] ----------------
            w1sb = pp.tile([128, 4, 40], F32)
            nc.sync.dma_start(w1sb[:], w1e[:].rearrange("(k p) n -> p k n", p=128))
            ident = pp.tile([128, 128], F32)
            from concourse.masks import make_identity
            make_identity(nc, ident[:])

            NT = 512  # nodes per matmul tile
            mp_cm = tc.tile_pool(name="mm", bufs=3)
            mp = mp_cm.__enter__()
            psp_cm = tc.tile_pool(name="mmpa", bufs=2, space="PSUM"); psp = psp_cm.__enter__()
            for t0 in range(0, c.NPAD, NT):
                nt = min(NT, c.NPAD - t0)
                xtile = mp.tile([128, 4, NT], F32, tag="xt")
                nc.sync.dma_start(xtile[:, :, :nt],
                                  xT[:, t0:t0 + nt].rearrange("(k p) n -> p k n", p=128))
                ps = psp.tile([40, NT], F32, tag="mm1")
                for k in range(4):
                    nc.tensor.matmul(ps[:, :nt], w1sb[:, k, :], xtile[:, k, :nt],
                                     start=(k == 0), stop=(k == 3))
                xpsT = mp.tile([40, NT], F32, tag="xpsT")
                nc.scalar.copy(xpsT[:, :nt], ps[:, :nt])
                # transpose per 128-node chunk -> node-major
                for s0 in range(0, nt, 128):
                    b = (t0 + s0) // 128
                    pst = psp.tile([128, 40], F32, tag="tr1")
                    nc.tensor.transpose(pst[:], xpsT[:, s0:s0 + 128], ident[:40, :40])
                    nm = wp.tile([128, 40], F32, tag="nm")
                    nc.scalar.copy(nm[:], pst[:])
                    # nm layout = [a_s(4) | xp(32) | a_d(4)] (W1ext order)
                    # pad nodes get a_s = -1e30 (additive mask input)
                    nc.vector.tensor_add(
                        nm[:, 0:4], nm[:, 0:4],
                        pnt[:, b:b + 1].broadcast_to([128, 4]))
                    # a_d -> resident; compact row [a_s | xp] in one copy
                    nc.vector.tensor_copy(a_d1[:, b * 4:(b + 1) * 4], nm[:, 36:40])
                    nc.vector.tensor_copy(
                        comp1[:, b * c.ROW1C:(b + 1) * c.ROW1C], nm[:, 0:36])

            psp_cm.__exit__(None, None, None)
            mp_cm.__exit__(None, None, None)
            # write compact slice (partition-major) + allgather + expand
            nc.sync.dma_start(
                tc1_in[:].rearrange("(p w) -> p w", p=128), comp1[:])
            nc.gpsimd.collective_compute(
                "AllGather", ALU.bypass,
                ins=[tc1_in[:]], outs=[tc1_full[:]],
                replica_groups=replica_groups,
            )
            _expand_table(nc, tc, wp, cfg, tc1_full, tbl1, c.ROW1C)

            # ---------------- L1 edge phase (epilogue interleaved) --------
            gp_cm = tc.tile_pool(name="gat", bufs=5)
            gp = gp_cm.__enter__()
            tp_cm = tc.tile_pool(name="tmp", bufs=2)
            tpool = tp_cm.__enter__()
            w2sb = pp.tile([32, 12], F32)
            nc.sync.dma_start(w2sb[:], w2e[:])
            psp_cm = tc.tile_pool(name="mmpb", bufs=2, space="PSUM")
            psp = psp_cm.__enter__()

            def epi1(b):
                """h = elu(out/S + b1); xps2 = h @ W2ext; stash compact row."""
                S = outcat[:, b * 36:b * 36 + 4]
                nc.vector.tensor_add(
                    S, S, pot[:, b:b + 1].broadcast_to([128, 4]))
                ou = outcat[:, b * 36 + 4:(b + 1) * 36]
                r = wp.tile([128, 4], F32, tag="r1")
                nc.vector.reciprocal(r[:], S)
                z = wp.tile([128, 32], F32, tag="z")
                nc.vector.tensor_tensor(
                    out=z[:].rearrange("p (h c) -> p h c", h=4),
                    in0=ou.rearrange("p (h c) -> p h c", h=4),
                    in1=r[:, :, None].broadcast_to([128, 4, 8]),
                    op=ALU.mult)
                nc.vector.tensor_add(z[:], z[:], b1t[:])
                # elu: h = max(z,0) + exp(min(z,0)) - 1
                mneg = wp.tile([128, 32], F32, tag="mneg")
                nc.vector.tensor_scalar(out=mneg[:], in0=z[:], scalar1=0.0,
                                        scalar2=None, op0=ALU.min)
                q = wp.tile([128, 32], F32, tag="q")
                nc.scalar.activation(q[:], mneg[:], ACTF.Exp)
                h = hcat[:, b * 32:(b + 1) * 32]
                nc.vector.tensor_scalar(out=h, in0=z[:], scalar1=0.0,
                                        scalar2=None, op0=ALU.max)
                nc.vector.tensor_add(h, h, q[:])
                nc.vector.tensor_scalar_add(h, h, -1.0)
                # xps2 = h @ W2ext : transpose h -> [32, 128]
                psh = psp.tile([32, 128], F32, tag="trh")
                nc.tensor.transpose(psh[:], h, ident[:])
                hT = wp.tile([32, 128], F32, tag="hT")
                nc.scalar.copy(hT[:], psh[:])
                ps2 = psp.tile([12, 128], F32, tag="mm2")
                nc.tensor.matmul(ps2[:], w2sb[:], hT[:], start=True, stop=True)
                x2T = wp.tile([12, 128], F32, tag="x2T")
                nc.scalar.copy(x2T[:], ps2[:])
                ps3 = psp.tile([128, 12], F32, tag="tr2")
                nc.tensor.transpose(ps3[:], x2T[:], ident[:12, :12])
                nm2 = wp.tile([128, 12], F32, tag="nm2")
                nc.scalar.copy(nm2[:], ps3[:])
                # nm2 layout = [a_s2 | xp2(8) | a_d2 | pad] (W2ext order)
                # small ops on the near-idle ACT engine (DVE queue is the
                # contended resource during the edge phase)
                nc.scalar.add(nm2[:, 0:1], nm2[:, 0:1], pnt[:, b:b + 1])
                nc.scalar.copy(a_d2[:, b:b + 1], nm2[:, 9:10])
                nc.scalar.copy(comp2[:, b * 9:(b + 1) * 9], nm2[:, 0:9])

            _edge_layer(nc, tc, cfg, plan, gp, gip, wp, tpool, idxt, tbl1,
                        a_d1, outcat, layer=1, nit=nit, epi_cb=epi1)

            psp_cm.__exit__(None, None, None)
            nc.sync.dma_start(
                tc2_in[:].rearrange("(p w) -> p w", p=128), comp2[:])
            nc.gpsimd.collective_compute(
                "AllGather", ALU.bypass,
                ins=[tc2_in[:]], outs=[tc2_full[:]],
                replica_groups=replica_groups,
            )
            _expand_table(nc, tc, wp, cfg, tc2_full, tbl2, c.ROW2C)

            # ---------------- L2 edge phase (final epilogue interleaved) --
            def epi2(b):
                S2 = out2cat[:, b * 9:b * 9 + 1]
                nc.vector.tensor_add(S2, S2, pot[:, b:b + 1])
                ou2 = out2cat[:, b * 9 + 1:(b + 1) * 9]
                r2 = wp.tile([128, 1], F32, tag="r2")
                nc.vector.reciprocal(r2[:], S2)
                fo = wp.tile([128, 8], F32, tag="fo")
                nc.vector.tensor_scalar(out=fo[:], in0=ou2, scalar1=r2[:],
                                        scalar2=None, op0=ALU.mult)
                nc.vector.tensor_add(fo[:], fo[:], b2t[:])
                nc.sync.dma_start(y[:, b * 8:(b + 1) * 8], fo[:])

            _edge_layer(nc, tc, cfg, plan, gp, gip, wp, tpool, idxt, tbl2,
                        a_d2, out2cat, layer=2, nit=nit, epi_cb=epi2)
            tp_cm.__exit__(None, None, None)
            gp_cm.__exit__(None, None, None)

    nc.finalize()
    return nc



def _dma_gather_raw(gps, out_ap, in_ap, idxs_ap, num_idxs, elem_size,
                    elem_step, queue_num, num_idxs_reg=None):
    """bass.BassGpSimd.dma_gather with the elem_size%256 assert relaxed to %4
    (the Q7 ucode handles arbitrary element lengths; verified on HW).

    num_idxs_reg: optional dynamic count (<= num_idxs). Must equal the
    post-trim count (trailing negative idxs) so the decode-side ring
    bookkeeping stays in lockstep with the Q7 descriptor pushes."""
    from concourse import ap_utils
    from concourse.bass import MemorySpace
    import concourse.mybir as mb

    assert idxs_ap.dtype == I16
    assert in_ap.dtype == out_ap.dtype
    elem_size_bytes = elem_size * mb.dt.size(in_ap.dtype)
    assert elem_size_bytes > 0 and elem_size_bytes % 4 == 0
    assert in_ap.space == MemorySpace.DRAM
    assert idxs_ap.space == MemorySpace.SBUF
    assert out_ap.space == MemorySpace.SBUF
    assert ap_utils.ap_is_contiguous(out_ap.ap[1:])
    assert ap_utils.ap_is_contiguous(idxs_ap.ap[1:])
    assert in_ap.ap[-1][1] == out_ap.ap[-1][1] == elem_size
    assert out_ap.ap[0][1] * out_ap.ap[1][1] == ((num_idxs + 127) // 128) * 128
    assert in_ap.ap[0][0] == elem_step
    stride_bytes = elem_step * mb.dt.size(in_ap.dtype)
    assert stride_bytes % 256 == 0
    stride_bytes_256 = stride_bytes // 256
    assert stride_bytes_256 < 256

    _in_ap = gps.lower_ap_dma(in_ap, for_custom_bir_dma=True)
    _idxs_ap = gps.lower_ap(idxs_ap)
    _out_ap = gps.lower_ap(out_ap)
    if num_idxs_reg is None:
        num_idxs_reg = num_idxs
    return gps.add_instruction(
        mb.InstDMAGatherAnt(
            name=gps.bass.get_next_instruction_name(),
            ins=[*_in_ap, _idxs_ap,
                 gps.lower_val_access(gps.to_reg(num_idxs_reg))],
            outs=[_out_ap],
            transpose=False,
            num_idxs=num_idxs,
            elem_size=elem_size,
            stride_bytes_256=stride_bytes_256,
            gen_mode=0,
            single_packet=False,
            queue_num=queue_num,
        )
    )


def _expand_table(nc, tc, wp_unused, cfg, compact_dram, padded_dram, roww):
    """Expand compact rows [TROWS, roww] (flat) to 256B rows [TROWS, 64].
    Group-ordered: each int16-addressable table quarter is expanded in
    sequence (full 128-partition width within the quarter), so group-g
    gathers can begin as soon as quarter g is written."""
    c = cfg
    GR = c.GROUP_ROWS              # rows per group (25088)
    assert GR % 128 == 0
    rpp = GR // 128                # rows per partition within a group
    CH = 4
    while rpp % CH != 0:
        CH -= 1
    rch = rpp // CH
    ep_cm = tc.tile_pool(name=f"exp{roww}", bufs=2)
    ep = ep_cm.__enter__()
    for g in range(c.GROUPS):
        srcg = compact_dram[g * GR * roww:(g + 1) * GR * roww].rearrange(
            "(p r w) -> p r w", p=128, w=roww)
        dstg = padded_dram[g * GR:(g + 1) * GR, :].rearrange(
            "(p r) w -> p r w", p=128)
        for ch in range(CH):
            ct = ep.tile([128, rch, roww], F32, tag="exp_in")
            nc.sync.dma_start(ct[:], srcg[:, ch * rch:(ch + 1) * rch, :])
            # full 256B rows: partial-row writes cost HBM read-modify-write
            pt = ep.tile([128, rch, c.ROWP], F32, tag="exp_out")
            nc.vector.memset(pt[:, :, roww:], 0.0)
            nc.vector.tensor_copy(pt[:, :, :roww], ct[:])
            nc.sync.dma_start(dstg[:, ch * rch:(ch + 1) * rch, :], pt[:])
    ep_cm.__exit__(None, None, None)


def _edge_layer(nc, tc, cfg, plan, gp, gip, wp, tpool, idxt, tbl, a_d, outcat,
                layer, nit=None, epi_cb=None, lag=2):
    """Edge phase: per (block, group) gather + attention + aggregation."""
    c = cfg
    H = c.H1 if layer == 1 else c.H2        # heads
    CC = c.C1 if layer == 1 else c.C2       # channels/head
    aw = 4 if layer == 1 else 1             # a_s words at row start
    xw = H * CC                             # xp words
    GP_BUFS = 5
    idx_off = 0
    Dmax = int(plan.D.sum(1).max())
    nregs = None
    if USE_TRIM and nit is not None:
        nregs = [nc.gpsimd.alloc_register(f"nidx_l{layer}_q{g}")
                 for g in range(c.GROUPS)]
    for b in range(c.NBLK):
        Dt = int(plan.D[b].sum())           # total slots/partition this block
        RW = 4 + xw                        # gathered words per row
        Gf = gp.tile([128, Dmax, RW], F32, tag=f"G{layer}")
        G = Gf[:, :Dt, :]
        if b < GP_BUFS:
            # first rotation: clear stale SBUF (NaN-safe: pv=0 * garbage)
            nc.vector.memset(Gf[:], 0.0)
        # one idx DMA per block (group segments are adjacent in idxt)
        itf = gip.tile([128, 8 * Dmax], I16, tag="it")
        itb = itf[:, :8 * Dt]
        nc.sync.dma_start(itb[:], idxt[:, idx_off:idx_off + 8 * Dt])
        idx_off += 8 * Dt
        off = 0
        for g in range(c.GROUPS):
            Dg = int(plan.D[b, g])
            Dmin_g = int(plan.Dmin[b, g])
            if Dmin_g < Dg:
                # columns this core may trim: force p = 0 via a_s = -inf
                nc.vector.memset(G[:, off + Dmin_g:off + Dg, 0:aw], NEG_BIG)
            nsl = 128 * Dg
            nreg = None
            if nregs is not None:
                k = b * c.GROUPS + g
                nreg = nregs[g]
                nc.gpsimd.reg_load(nreg, nit[0:1, k:k + 1])
            _dma_gather_raw(
                nc.gpsimd,
                G[:, off:off + Dg, :],
                tbl[g * c.GROUP_ROWS:(g + 1) * c.GROUP_ROWS, :RW],
                itb[:, 8 * off:8 * (off + Dg)], nsl, RW, c.ROWP,
                queue_num=g % 4, num_idxs_reg=nreg,
            )
            off += Dg
        Hm = cfg.H1
        if USE_ACTFUSE:
            # lr = leaky_relu(a_s + a_d) fused on the ACT engine
            lrf = wp.tile([128, Hm, Dmax], F32, tag="lr")
            lr = lrf[:, :H, :Dt]
            for h in range(H):
                nc.scalar.activation(
                    lr[:, h, :], G[:, :, h], ACTF.Prelu,
                    bias=a_d[:, b * H + h:b * H + h + 1], scale=1.0,
                    alpha=c.NEG_SLOPE)
            # p = exp(lr); S = sum_j p via the ACT accumulator
            pvf = wp.tile([128, Hm, Dmax], F32, tag="p")
            pv = pvf[:, :H, :Dt]
            for h in range(H):
                nc.scalar.activation(
                    pv[:, h, :], lr[:, h, :], ACTF.Exp,
                    accum_out=outcat[:, b * (H + xw) + h:b * (H + xw) + h + 1])
        else:
            epf = wp.tile([128, Hm, Dmax], F32, tag="e")
            ep = epf[:, :H, :Dt]
            for h in range(H):
                nc.scalar.activation(
                    ep[:, h, :], G[:, :, h], ACTF.Identity,
                    bias=a_d[:, b * H + h:b * H + h + 1], scale=1.0)
            lrf = wp.tile([128, Hm, Dmax], F32, tag="lr")
            lr = lrf[:, :H, :Dt]
            nc.vector.tensor_scalar(out=lr[:], in0=ep[:], scalar1=0.0,
                                    scalar2=c.NEG_SLOPE, op0=ALU.min,
                                    op1=ALU.mult)
            pposf = wp.tile([128, Hm, Dmax], F32, tag="ppos")
            ppos = pposf[:, :H, :Dt]
            nc.vector.tensor_scalar(out=ppos[:], in0=ep[:], scalar1=0.0,
                                    scalar2=None, op0=ALU.max)
            nc.vector.tensor_add(lr[:], lr[:], ppos[:])
            pvf = wp.tile([128, Hm, Dmax], F32, tag="p")
            pv = pvf[:, :H, :Dt]
            nc.scalar.activation(pv[:], lr[:], ACTF.Exp)
            nc.vector.tensor_reduce(
                out=outcat[:, b * (H + xw):b * (H + xw) + H],
                in_=pv[:], op=ALU.add, axis=AX)
        # msg = p (bcast over CC) * xp ; out_un = sum_j msg
        tmpf = tpool.tile([128, c.H1 * c.C1, Dmax], F32, tag="tmp")
        tmp = tmpf[:, :H * CC, :Dt]
        if H > 1:
            nc.vector.tensor_tensor(
                out=tmp[:].rearrange("p (h c) d -> p h c d", h=H),
                in0=pv[:, :, None, :].broadcast_to([128, H, CC, Dt]),
                in1=G[:, :, aw:aw + xw].rearrange("p d (h c) -> p h c d", h=H),
                op=ALU.mult)
        else:
            # 3D form: a size-1 head dim lowers to a pathologically slow
            # DVE instruction
            nc.vector.tensor_tensor(
                out=tmp[:],
                in0=pv[:, 0, None, :].broadcast_to([128, CC, Dt]),
                in1=G[:, :, aw:aw + xw].rearrange("p d c -> p c d"),
                op=ALU.mult)
        nc.vector.tensor_reduce(
            out=outcat[:, b * (H + xw) + H:(b + 1) * (H + xw)],
            in_=tmp[:], op=ALU.add, axis=AX)
        # interleave the per-block epilogue under the (desc-gen-bound)
        # edge phase so it rides in the engine-queue shadow
        if epi_cb is not None and b - lag >= 0:
            epi_cb(b - lag)
    if epi_cb is not None:
        for bb in range(max(c.NBLK - lag, 0), c.NBLK):
            epi_cb(bb)


# ----------------------------------------------------------------------------
# Host wrapper
# ----------------------------------------------------------------------------
def _build_w1ext(W1, att_src1, att_dst1):
    # [W1@As | W1 | W1@Ad]: As[j, h] = att_src1[h, j%C] if j//C==h
    H, C = att_src1.shape
    As = np.zeros((H * C, H), np.float32)
    Ad = np.zeros((H * C, H), np.float32)
    for h in range(H):
        As[h * C:(h + 1) * C, h] = att_src1[h]
        Ad[h * C:(h + 1) * C, h] = att_dst1[h]
    return np.concatenate([W1 @ As, W1, W1 @ Ad], axis=1).astype(np.float32)


def _build_w2ext(W2, att_src2, att_dst2):
    H, C = att_src2.shape
    As = att_src2.reshape(C, 1).astype(np.float32)
    Ad = att_dst2.reshape(C, 1).astype(np.float32)
    out = np.concatenate([W2 @ As, W2, W2 @ Ad, np.zeros((32, 2), np.float32)],
                         axis=1)
    return out.astype(np.float32)


def _pad_masks(cfg, node_at):
    """[128, NB] additive masks: NEG_BIG / 1.0 on pad (rank) positions."""
    c = cfg
    is_pad = (node_at < 0).reshape(c.NBLK, 128).T  # [128, NB]
    neg = np.where(is_pad, np.float32(NEG_BIG), np.float32(0.0))
    one = np.where(is_pad, np.float32(1.0), np.float32(0.0))
    return np.ascontiguousarray(neg), np.ascontiguousarray(one)


LAST_EXEC_NS = None


def kernel(x, edge_index, W1, att_src1, att_dst1, b1, W2, att_src2, att_dst2,
           b2):
    cfg = Cfg(N=x.shape[0], E=edge_index.shape[1], F_IN=x.shape[1])
    plan = Plan(cfg, np.asarray(edge_index))
    nc = build_kernel(cfg, plan)

    x = np.asarray(x, dtype=np.float32)
    w1e = _build_w1ext(np.asarray(W1), np.asarray(att_src1), np.asarray(att_dst1))
    w2e = _build_w2ext(np.asarray(W2), np.asarray(att_src2), np.asarray(att_dst2))
    b1r = np.tile(np.asarray(b1, np.float32)[None, :], (128, 1))
    b2r = np.tile(np.asarray(b2, np.float32)[None, :], (128, 1))

    in_maps = []
    for ci in range(cfg.NCORES):
        na = plan.node_at[ci]
        xs = np.zeros((cfg.NPAD, cfg.F_IN), np.float32)
        realm = na >= 0
        xs[realm] = x[ci * cfg.NPC:(ci + 1) * cfg.NPC][na[realm]]
        neg, one = _pad_masks(cfg, na)
        nidx = (128 * plan.Dk[ci].reshape(1, -1)).astype(np.int32)
        if not USE_TRIM:
            nidx = (128 * np.broadcast_to(
                plan.D.reshape(1, -1), nidx.shape)).astype(np.int32)
        in_maps.append({
            "xT": np.ascontiguousarray(xs.T),
            "w1e": w1e, "w2e": w2e, "b1r": b1r, "b2r": b2r,
            "padneg": neg, "padone": one,
            "idxt": plan.idx_planes[ci],
            "nidx": nidx,
        })

    global LAST_EXEC_NS
    want_trace = False
    try:
        from antenv.axon_hooks import get_axon_ntff_profile_hook
        want_trace = get_axon_ntff_profile_hook() is not None
    except ImportError:
        pass
    res = run_bass_kernel_spmd(nc, in_maps, core_ids=list(range(cfg.NCORES)),
                               trace=want_trace)
    LAST_EXEC_NS = res.exec_time_ns

    out = np.empty((cfg.N, 8), np.float32)
    for ci in range(cfg.NCORES):
        yv = res.results[ci]["y"].reshape(128, cfg.NBLK, 8)
        na = plan.node_at[ci]
        ranks = np.arange(cfg.NPAD)
        realm = na >= 0
        out[ci * cfg.NPC + na[realm]] = yv[ranks[realm] % 128,
                                           ranks[realm] // 128, :]
    return out


if __name__ == "__main__":
    pass
